# revision 1
# baseline (speedup 1.0000x reference)
"""MinGRU Trainium2 kernel.

Full-input contract: kernel(x=[8,4096,1024] f32, W_hg=[2048,1024] f32)
-> [8,4096,1024] f32.

Sharding: data-parallel over batch. Core i computes example i entirely
(matmul + pointwise + sequential scan along seq); W_hg is replicated.

Math (equivalent to the log-space reference, but computed in linear space,
which is stable here because a_t = sigmoid(-gate) is in (0,1) and every
summand is positive):
    hg     = x @ W_hg.T ; hidden, gate = split(hg)
    a_t    = sigmoid(-gate_t)                        # 1 - z_t
    g~_t   = min(sigmoid(hidden_t), 0.5) + relu(hidden_t)
    b_t    = sigmoid(gate_t) * g~_t
    h_t    = a_t * h_{t-1} + b_t                     # tensor_tensor_scan

Device layout: channels on partitions, seq on the free dim. The host
pre-transposes x[i] -> xT [D, S] and W_hg -> wT [D, 2D] so no on-chip
transposes are needed and the fp32r matmul result lands scan-ready.

Measured on trn2 (marginal cost of extra For_i passes, min-of-12):
~210 us/pass per core -- at the PE fp32r streaming floor (~218 us for
16 e-blocks x 8 k-tiles x 4096 columns @ 2.4 GHz). A seq-chunk-outer
variant with W resident and x streamed modeled better (256 vs 286 us in
the cost model) but measured worse on hardware (293 us/pass), so this
d-block-outer, x-resident structure is kept.
"""

from contextlib import ExitStack

import numpy as np

B, S, D = 8, 4096, 1024
E = 2 * D
P = 128
KT = D // P  # contraction k-tiles
DB = D // P  # output-channel blocks
SC = 512  # seq chunk (PSUM bank = 512 f32)
NSC = S // SC

_NC_CACHE = {}


def _build_bass(repeat=1, loop_repeat=None, psum_bufs=2):
    import contextlib

    import concourse.tile as tile
    from concourse import bacc, mybir

    f32 = mybir.dt.float32
    f32r = mybir.dt.float32r
    AF = mybir.ActivationFunctionType
    OP = mybir.AluOpType

    nc = bacc.Bacc("TRN2", debug=False)
    xT = nc.dram_tensor("xT", [D, S], f32r, kind="ExternalInput").ap()
    wT = nc.dram_tensor("wT", [D, E], f32r, kind="ExternalInput").ap()
    out = nc.dram_tensor("out", [D, S], f32, kind="ExternalOutput").ap()

    # [k, p, e] view of wT for one-shot strided weight-slice loads
    wT_k = wT.rearrange("(k p) e -> p k e", p=P)

    with tile.TileContext(nc) as tc, ExitStack() as ctx:
        xpool = ctx.enter_context(tc.tile_pool(name="x", bufs=1))
        wpool = ctx.enter_context(tc.tile_pool(name="w", bufs=2))
        ppool = ctx.enter_context(
            tc.tile_pool(name="ps", bufs=psum_bufs, space="PSUM")
        )
        spool = ctx.enter_context(tc.tile_pool(name="s", bufs=2))
        opool = ctx.enter_context(tc.tile_pool(name="o", bufs=4))

        loop_cm = (
            tc.For_i(0, loop_repeat, 1)
            if loop_repeat is not None
            else contextlib.nullcontext()
        )
        with loop_cm:
            for _rep in range(repeat):
                # x fully resident: 64 tiles [128, 512], loaded seq-chunk-major
                # so the first d-block's first matmuls start after ~2MB of DMA.
                xt = [[None] * NSC for _ in range(KT)]
                for sc in range(NSC):
                    for k in range(KT):
                        t = xpool.tile([P, SC], f32r, tag=f"x{k}_{sc}")
                        nc.sync.dma_start(
                            t[:], xT[k * P : (k + 1) * P, sc * SC : (sc + 1) * SC]
                        )
                        xt[k][sc] = t

                for db in range(DB):
                    eh = db * P  # hidden channel block
                    eg = D + db * P  # gate channel block
                    wh = wpool.tile([P, KT, P], f32r, tag="wh")
                    nc.sync.dma_start(wh[:], wT_k[:, :, eh : eh + P])
                    wg = wpool.tile([P, KT, P], f32r, tag="wg")
                    nc.sync.dma_start(wg[:], wT_k[:, :, eg : eg + P])

                    prev_o = None
                    for sc in range(NSC):
                        ph = ppool.tile([P, SC], f32, tag="ph")
                        pg = ppool.tile([P, SC], f32, tag="pg")
                        for k in range(KT):
                            nc.tensor.matmul(
                                ph[:],
                                wh[:, k, :],
                                xt[k][sc][:],
                                start=(k == 0),
                                stop=(k == KT - 1),
                            )
                        for k in range(KT):
                            nc.tensor.matmul(
                                pg[:],
                                wg[:, k, :],
                                xt[k][sc][:],
                                start=(k == 0),
                                stop=(k == KT - 1),
                            )

                        # ScalarE straight out of PSUM
                        a = spool.tile([P, SC], f32, tag="a")
                        nc.scalar.activation(a[:], pg[:], AF.Sigmoid, scale=-1.0)
                        z = spool.tile([P, SC], f32, tag="z")
                        nc.scalar.activation(z[:], pg[:], AF.Sigmoid)
                        sh = spool.tile([P, SC], f32, tag="sh")
                        nc.scalar.activation(sh[:], ph[:], AF.Sigmoid)
                        r = spool.tile([P, SC], f32, tag="r")
                        nc.scalar.activation(r[:], ph[:], AF.Relu)

                        # g~ = min(sigmoid(h), 0.5) + relu(h);  b = z * g~
                        gt = spool.tile([P, SC], f32, tag="gt")
                        nc.vector.scalar_tensor_tensor(
                            gt[:], sh[:], 0.5, r[:], op0=OP.min, op1=OP.add
                        )
                        b = spool.tile([P, SC], f32, tag="b")
                        nc.vector.tensor_mul(b[:], z[:], gt[:])

                        o = opool.tile([P, SC], f32, tag="o")
                        init = 0.0 if sc == 0 else prev_o[:, SC - 1 : SC]
                        nc.vector.tensor_tensor_scan(
                            o[:], a[:], b[:], init, op0=OP.mult, op1=OP.add
                        )
                        prev_o = o
                        nc.sync.dma_start(
                            out[db * P : (db + 1) * P, sc * SC : (sc + 1) * SC],
                            o[:],
                        )
    nc.compile()
    return nc


def _get_nc():
    if "nc" not in _NC_CACHE:
        _NC_CACHE["nc"] = _build_bass()
    return _NC_CACHE["nc"]


def _run(in_maps, trace=False, **kw):
    from concourse import bass_utils

    nc = _get_nc()
    return bass_utils.run_bass_kernel_spmd(
        nc, in_maps, core_ids=list(range(B)), trace=trace, **kw
    )


def _make_in_maps(x, W_hg):
    x = np.ascontiguousarray(x, dtype=np.float32)
    wT = np.ascontiguousarray(W_hg.T, dtype=np.float32)
    return [
        {"xT": np.ascontiguousarray(x[i].T), "wT": wT} for i in range(B)
    ]


def kernel(x, W_hg):
    res = _run(_make_in_maps(x, W_hg))
    outs = [r["out"] for r in res.results]
    return np.stack([o.T for o in outs], axis=0).astype(np.float32)

